# revision 11
# baseline (speedup 1.0000x reference)
"""Trainium2 Bass kernel for nn_Dendrite_755914244697.

Computation (per output element [c, oh, ow, n]):
    t[ij]  = x[c, oh+i, ow+j] * w[c,oh,ow,n,i,j] - q[c,oh,ow,n,i,j]
    u[ij]  = arctan(10*t[ij])                     (u in (-pi/2, pi/2))
    z[ij]  = 1.1 + u[ij]/pi                       (z in (0.6, 1.6), > 0)
    out    = sum_ij ln(z[ij])

Two equivalent evaluation paths are blended to balance the Scalar (ACT)
and Vector (DVE) engines:
  A (sum path, n in [0, NSUM)):   out = sum_ij ln(u/pi + 1.1)
      ACT ln on every element (the affine folds into ln's input scale/bias),
      then one DVE sum-reduce over ij.
  B (prod path, n in [NSUM, 25)): out = ln(pi^-25 * prod_ij (u + 1.1*pi))
      DVE add-const + two product-reduces (over j then i), then one tiny
      ACT ln on 1/25 of the elements (scale folds the pi^-25).
The atan (full size) runs on ACT for both paths. ACT work ~ atan + NSUM/25
of the ln; DVE work grows with the B fraction. NSUM tunes the balance.

I/O is fp16 (tolerance is 2e-2; fp16 keeps norm-rel error ~1e-3) which
halves HBM traffic and enables DVE 2x/4x modes. The big w/q loads are
issued on the gpsimd SWDGE queue (engages all 16 SDMA engines; the sync
HWDGE queue alone only sustains ~80 GB/s) with p/outputs on the HWDGE
queues.

Sharding: out-height split across 8 cores (16 rows each, tail zero-padded),
per the spatial-parallel hint; x's halo is resolved on host by pre-
extracting the 5x5 patches each core needs.

Device layout per core, per channel c (3 channels):
    SBUF tiles are [124 partitions = ow, free = (oh:16, n:25, ij:25)].
"""

import math
import os
import time

os.environ.setdefault("BASS_NEVER_TRACE", "1")

import numpy as np

NCORES = 8
ROWS = 16          # oh rows per core (8*16 = 128 >= 124, tail zero-padded)
OUT = 124          # spatial out dim (and #partitions = ow)
NUM = 25
SIDE = 5
IJ = SIDE * SIDE   # 25 window positions
CH = 3
F = ROWS * NUM * IJ  # free elems per channel tile (10000)

NSUM = 11          # n in [0, NSUM) -> sum path; [NSUM, 25) -> prod path
NPROD = NUM - NSUM

IO_DTYPE = "float16"

# queue assignment experiment: which engine issues each big load
# (g=gpsimd SWDGE, s=sync HWDGE, a=scalar/ACT HWDGE)
W_QUEUE = ["g", "s", "g"]
Q_QUEUE = ["g", "a", "a"]

_PROGRAM = None


def _build_program():
    import concourse.bacc as bacc
    import concourse.tile as tile
    import concourse.mybir as mybir

    nc = bacc.Bacc(
        "TRN2",
        target_bir_lowering=False,
        debug=False,
        enable_asserts=False,
        num_devices=NCORES,
    )
    dt = getattr(mybir.dt, IO_DTYPE)
    f32 = mybir.dt.float32
    AF = mybir.ActivationFunctionType
    ALU = mybir.AluOpType

    # (OUT, CH*F): one 60 KB row per ow partition -> max-size DMA descriptors
    wt = nc.dram_tensor("wt", (OUT, CH * F), dt, kind="ExternalInput")
    qt = nc.dram_tensor("qt", (OUT, CH * F), dt, kind="ExternalInput")
    pt = nc.dram_tensor("pt", (CH, OUT, ROWS * IJ), dt, kind="ExternalInput")
    ot = nc.dram_tensor("ot", (CH, OUT, ROWS * NUM), f32, kind="ExternalOutput")

    C_ADD = 1.1 * math.pi          # z*pi = u + 1.1*pi
    LN_SCALE = math.pi ** (-NUM)   # ln(prod(z*pi) * pi^-25) = sum ln z

    eng = {"g": None, "s": None, "a": None}

    with tile.TileContext(nc) as tc:
        eng = {"g": nc.gpsimd, "s": nc.sync, "a": nc.scalar}
        with (
            tc.tile_pool(name="wp", bufs=1) as wp,
            tc.tile_pool(name="qp", bufs=1) as qp,
            tc.tile_pool(name="pp", bufs=2) as pp,
            tc.tile_pool(name="bp", bufs=2) as bp,
            tc.tile_pool(name="rp", bufs=2) as rp,
            tc.tile_pool(name="op", bufs=1) as op,
        ):
            bias_t = op.tile([OUT, 1], f32, tag="bias", name="bias")
            nc.vector.memset(bias_t[:], 1.1)
            # Single max-descriptor loads: w via gpsimd SWDGE, q via sync HWDGE
            w_all = wp.tile([OUT, CH * F], dt, tag="w", name="w_all")
            nc.gpsimd.dma_start(w_all[:], wt.ap())
            q_all = qp.tile([OUT, CH * F], dt, tag="q", name="q_all")
            nc.sync.dma_start(q_all[:], qt.ap())
            w_t = [w_all[:, c * F : (c + 1) * F] for c in range(CH)]
            q_t = [q_all[:, c * F : (c + 1) * F] for c in range(CH)]
            p_t, o_t = [], []
            for c in range(CH):
                p_ = pp.tile([OUT, ROWS * IJ], dt, tag="p")
                nc.scalar.dma_start(p_[:], pt.ap()[c])
                p_t.append(p_)
                o_t.append(
                    op.tile([OUT, ROWS * NUM], f32, tag=f"o{c}", name=f"o{c}")
                )

            # DVE: t = p*w - q, in place in the w tile
            for c in range(CH):
                w4 = w_t[c].rearrange("p (a n c) -> p a n c", a=ROWS, n=NUM)
                p4 = (
                    p_t[c][:]
                    .rearrange("p (a c) -> p a c", a=ROWS)
                    .unsqueeze(2)
                    .broadcast_to((OUT, ROWS, NUM, IJ))
                )
                nc.vector.tensor_mul(w4, p4, w4)
                nc.vector.tensor_sub(w_t[c], w_t[c], q_t[c])

            # ACT: u = arctan(10*t) in place (one table load for all three)
            for c in range(CH):
                nc.scalar.activation(
                    w_t[c], w_t[c], AF.Arctan, bias=0.0, scale=10.0
                )

            with nc.allow_low_precision(reason="fp16 pipeline; tol 2e-2"):
                for c in range(CH):
                    u4 = w_t[c].rearrange(
                        "p (a n c) -> p a n c", a=ROWS, n=NUM
                    )
                    o3 = o_t[c][:].rearrange("p (a n) -> p a n", a=ROWS)
                    # --- B (prod) path: DVE work first so it overlaps ---
                    ub = u4[:, :, NSUM:, :]
                    b_ = bp.tile([OUT, ROWS * NPROD * IJ], dt, tag="b")
                    b4 = b_[:].rearrange(
                        "p (a n c) -> p a n c", a=ROWS, n=NPROD
                    )
                    nc.vector.tensor_scalar_add(b4, ub, C_ADD)
                    # prod over j (innermost 5)
                    rj = rp.tile([OUT, ROWS * NPROD * SIDE], dt, tag="rj")
                    nc.vector.tensor_reduce(
                        rj[:],
                        b_[:].rearrange("p (g c) -> p g c", c=SIDE),
                        axis=mybir.AxisListType.X,
                        op=ALU.mult,
                    )
                    # prod over i (innermost 5), fp32 out
                    ri = rp.tile([OUT, ROWS * NPROD], f32, tag="ri")
                    nc.vector.tensor_reduce(
                        ri[:],
                        rj[:].rearrange("p (g c) -> p g c", c=SIDE),
                        axis=mybir.AxisListType.X,
                        op=ALU.mult,
                    )
                    # tiny ln: out = ln(pi^-25 * prod)
                    nc.scalar.activation(
                        o3[:, :, NSUM:],
                        ri[:].rearrange("p (a n) -> p a n", a=ROWS),
                        AF.Ln,
                        bias=0.0,
                        scale=LN_SCALE,
                    )
                    # --- A (sum) path ---
                    ua = u4[:, :, :NSUM, :]
                    nc.scalar.activation(
                        ua, ua, AF.Ln, bias=bias_t[:], scale=float(1.0 / math.pi)
                    )
                    nc.vector.tensor_reduce(
                        o3[:, :, :NSUM],
                        ua,
                        axis=mybir.AxisListType.X,
                        op=ALU.add,
                    )
                    nc.scalar.dma_start(ot.ap()[c], o_t[c][:])

    nc.compile()
    return nc


def _get_program():
    global _PROGRAM
    if _PROGRAM is None:
        _PROGRAM = _build_program()
    return _PROGRAM


def _prep_inputs(x, w, q):
    """Slice/transpose full inputs into 8 per-core input maps."""
    from numpy.lib.stride_tricks import sliding_window_view

    np_dt = np.dtype(IO_DTYPE)
    # patches[c, oh, ow, ij] = x[0, c, oh+i, ow+j]
    patches = sliding_window_view(x[0], (SIDE, SIDE), axis=(1, 2)).reshape(
        CH, OUT, OUT, IJ
    )
    w = w.reshape(CH, OUT, OUT, NUM * IJ)
    q = q.reshape(CH, OUT, OUT, NUM * IJ)

    in_maps = []
    for k in range(NCORES):
        r0 = k * ROWS
        r1 = min(r0 + ROWS, OUT)
        nr = r1 - r0

        wk = np.zeros((CH, OUT, ROWS, NUM * IJ), np_dt)
        wk[:, :, :nr, :] = w[:, r0:r1].transpose(0, 2, 1, 3)
        qk = np.zeros((CH, OUT, ROWS, NUM * IJ), np_dt)
        qk[:, :, :nr, :] = q[:, r0:r1].transpose(0, 2, 1, 3)
        pk = np.zeros((CH, OUT, ROWS, IJ), np_dt)
        pk[:, :, :nr, :] = patches[:, r0:r1].transpose(0, 2, 1, 3)
        in_maps.append(
            {
                "wt": wk.reshape(CH, OUT, F).transpose(1, 0, 2).reshape(OUT, CH * F),
                "qt": qk.reshape(CH, OUT, F).transpose(1, 0, 2).reshape(OUT, CH * F),
                "pt": pk.reshape(CH, OUT, ROWS * IJ),
            }
        )
    return in_maps


def _assemble_output(results):
    parts = []
    for k in range(NCORES):
        r0 = k * ROWS
        nr = min(r0 + ROWS, OUT) - r0
        ok = results[k]["ot"].reshape(CH, OUT, ROWS, NUM)
        parts.append(ok.transpose(0, 2, 1, 3)[:, :nr])  # (CH, nr, OUT, NUM)
    out = np.concatenate(parts, axis=1)  # (CH, OUT, OUT, NUM)
    return out[None].astype(np.float32)


def kernel(x, w, q):
    from concourse.bass_utils import run_bass_kernel_spmd

    nc = _get_program()
    in_maps = _prep_inputs(
        np.asarray(x, np.float32), np.asarray(w, np.float32), np.asarray(q, np.float32)
    )
    res = run_bass_kernel_spmd(nc, in_maps, list(range(NCORES)), trace=False)
    return _assemble_output(res.results)


def bench(x, w, q, iters=30):
    """Steady-state per-call wall time (ns) with device-resident inputs.

    Replicates bass2jax.run_bass_via_pjrt's multi-core path (shard_map over 8
    cores) but without output-buffer donation, so the jitted executable can be
    invoked repeatedly on the same device buffers.
    """
    import jax
    import numpy as _np
    from jax.sharding import Mesh, PartitionSpec
    from jax.experimental.shard_map import shard_map
    import concourse.mybir as mybir
    from concourse import bass2jax

    bass2jax.install_neuronx_cc_hook()
    nc = _get_program()
    in_maps = _prep_inputs(
        np.asarray(x, np.float32), np.asarray(w, np.float32), np.asarray(q, np.float32)
    )

    partition_name = nc.partition_id_tensor.name if nc.partition_id_tensor else None
    in_names, out_names, out_avals, zero_outs = [], [], [], []
    for alloc in nc.m.functions[0].allocations:
        if not isinstance(alloc, mybir.MemoryLocationSet):
            continue
        name = alloc.memorylocations[0].name
        if alloc.kind == "ExternalInput":
            if name != partition_name:
                in_names.append(name)
        elif alloc.kind == "ExternalOutput":
            out_names.append(name)
            shape = tuple(alloc.tensor_shape)
            dtype = mybir.dt.np(alloc.dtype)
            out_avals.append(jax.core.ShapedArray(shape, dtype))
            zero_outs.append(_np.zeros(shape, dtype))
    n_params = len(in_names)
    all_names = in_names + out_names
    if partition_name is not None:
        all_names = all_names + [partition_name]

    def _body(*args):
        operands = list(args)
        if partition_name is not None:
            operands.append(bass2jax.partition_id_tensor())
        outs = bass2jax._bass_exec_p.bind(
            *operands,
            out_avals=tuple(out_avals),
            in_names=tuple(all_names),
            out_names=tuple(out_names),
            lowering_input_output_aliases=(),
            sim_require_finite=True,
            sim_require_nnan=True,
            nc=nc,
        )
        return tuple(outs)

    devices = jax.devices()[:NCORES]
    mesh = Mesh(_np.asarray(devices), ("core",))
    nin = n_params + len(out_names)
    sharded = jax.jit(
        shard_map(
            _body,
            mesh=mesh,
            in_specs=(PartitionSpec("core"),) * nin,
            out_specs=(PartitionSpec("core"),) * len(out_names),
            check_rep=False,
        ),
        keep_unused=True,
    )
    concat_in = [
        _np.concatenate([in_maps[c][nm] for c in range(NCORES)], axis=0)
        for nm in in_names
    ]
    concat_zeros = [
        _np.zeros((NCORES * z.shape[0], *z.shape[1:]), z.dtype) for z in zero_outs
    ]
    args = [jax.device_put(a) for a in concat_in + concat_zeros]

    out = sharded(*args)  # compile + warmup
    jax.block_until_ready(out)
    times = []
    for _ in range(iters):
        t0 = time.perf_counter()
        out = sharded(*args)
        jax.block_until_ready(out)
        times.append(time.perf_counter() - t0)
    times.sort()
    med = times[len(times) // 2]
    print(
        f"bench: min {times[0] * 1e6:.1f} us, median {med * 1e6:.1f} us, "
        f"max {times[-1] * 1e6:.1f} us over {iters} iters"
    )
    return med * 1e9


# revision 12
# speedup vs baseline: 1.3880x; 1.3880x over previous
"""Trainium2 Bass kernel for nn_Dendrite_755914244697.

Computation (per output element [c, oh, ow, n]):
    t[ij]  = x[c, oh+i, ow+j] * w[c,oh,ow,n,i,j] - q[c,oh,ow,n,i,j]
    u[ij]  = arctan(10*t[ij])                     (u in (-pi/2, pi/2))
    z[ij]  = 1.1 + u[ij]/pi                       (z in (0.6, 1.6), > 0)
    out    = sum_ij ln(z[ij])

Two equivalent evaluation paths are blended to balance the Scalar (ACT)
and Vector (DVE) engines:
  A (sum path, n in [0, NSUM)):   out = sum_ij ln(u/pi + 1.1)
      ACT ln on every element (the affine folds into ln's input scale/bias),
      then one DVE sum-reduce over ij.
  B (prod path, n in [NSUM, 25)): out = ln(pi^-25 * prod_ij (u + 1.1*pi))
      DVE add-const + two product-reduces (over j then i), then one tiny
      ACT ln on 1/25 of the elements (scale folds the pi^-25).
The atan (full size) runs on ACT for both paths. ACT work ~ atan + NSUM/25
of the ln; DVE work grows with the B fraction. NSUM tunes the balance.

I/O is fp16 (tolerance is 2e-2; fp16 keeps norm-rel error ~1e-3) which
halves HBM traffic and enables DVE 2x/4x modes. The big w/q loads are
issued on the gpsimd SWDGE queue (engages all 16 SDMA engines; the sync
HWDGE queue alone only sustains ~80 GB/s) with p/outputs on the HWDGE
queues.

Sharding: out-height split across 8 cores (16 rows each, tail zero-padded),
per the spatial-parallel hint; x's halo is resolved on host by pre-
extracting the 5x5 patches each core needs.

Device layout per core, per channel c (3 channels):
    SBUF tiles are [124 partitions = ow, free = (oh:16, n:25, ij:25)].
"""

import math
import os
import time

os.environ.setdefault("BASS_NEVER_TRACE", "1")

import numpy as np

NCORES = 8
ROWS = 16          # oh rows per core (8*16 = 128 >= 124, tail zero-padded)
OUT = 124          # spatial out dim (and #partitions = ow)
NUM = 25
SIDE = 5
IJ = SIDE * SIDE   # 25 window positions
CH = 3
F = ROWS * NUM * IJ  # free elems per channel tile (10000)

IO_DTYPE = "float16"

_PROGRAM = None


CHUNKS = 2           # oh-halves per channel
HROWS = ROWS // CHUNKS
CF = HROWS * NUM * IJ  # 5000 free elems per chunk


def _build_program():
    import concourse.bacc as bacc
    import concourse.tile as tile
    import concourse.mybir as mybir

    nc = bacc.Bacc(
        "TRN2",
        target_bir_lowering=False,
        debug=False,
        enable_asserts=False,
        num_devices=NCORES,
    )
    dt = getattr(mybir.dt, IO_DTYPE)
    f32 = mybir.dt.float32
    AF = mybir.ActivationFunctionType
    ALU = mybir.AluOpType

    NCHUNK = CH * CHUNKS
    wt = nc.dram_tensor("wt", (NCHUNK, OUT, CF), dt, kind="ExternalInput")
    qt = nc.dram_tensor("qt", (NCHUNK, OUT, CF), dt, kind="ExternalInput")
    pt = nc.dram_tensor("pt", (NCHUNK, OUT, HROWS * IJ), dt, kind="ExternalInput")
    ot = nc.dram_tensor("ot", (NCHUNK, OUT, HROWS * NUM), dt, kind="ExternalOutput")

    with tile.TileContext(nc) as tc:
        with (
            tc.tile_pool(name="wp", bufs=4) as wp,
            tc.tile_pool(name="qp", bufs=4) as qp,
            tc.tile_pool(name="pp", bufs=4) as pp,
            tc.tile_pool(name="op", bufs=2) as op,
            tc.tile_pool(name="cp", bufs=1) as cp,
        ):
            bias_t = cp.tile([OUT, 1], f32, tag="bias", name="bias")
            nc.vector.memset(bias_t[:], 1.1)
            with nc.allow_low_precision(reason="fp16 pipeline; tol 2e-2"):
                for k in range(NCHUNK):
                    # all big loads on the gpsimd SWDGE ring: each in-flight
                    # dma_start gets its own 4 SDMA engines
                    w_ = wp.tile([OUT, CF], dt, tag="w")
                    nc.gpsimd.dma_start(w_[:], wt.ap()[k])
                    q_ = qp.tile([OUT, CF], dt, tag="q")
                    nc.gpsimd.dma_start(q_[:], qt.ap()[k])
                    p_ = pp.tile([OUT, HROWS * IJ], dt, tag="p")
                    nc.sync.dma_start(p_[:], pt.ap()[k])

                    w4 = w_[:].rearrange("p (a n c) -> p a n c", a=HROWS, n=NUM)
                    p4 = (
                        p_[:]
                        .rearrange("p (a c) -> p a c", a=HROWS)
                        .unsqueeze(2)
                        .broadcast_to((OUT, HROWS, NUM, IJ))
                    )
                    nc.vector.tensor_mul(w4, p4, w4)
                    nc.vector.tensor_sub(w_[:], w_[:], q_[:])
                    nc.scalar.activation(
                        w_[:], w_[:], AF.Arctan, bias=0.0, scale=10.0
                    )
                    nc.scalar.activation(
                        w_[:], w_[:], AF.Ln, bias=bias_t[:],
                        scale=float(1.0 / math.pi),
                    )
                    o_ = op.tile([OUT, HROWS * NUM], dt, tag="o")
                    nc.vector.tensor_reduce(
                        o_[:],
                        w_[:].rearrange("p (g c) -> p g c", c=IJ),
                        axis=mybir.AxisListType.X,
                        op=ALU.add,
                    )
                    nc.scalar.dma_start(ot.ap()[k], o_[:])

    nc.compile()
    return nc


def _get_program():
    global _PROGRAM
    if _PROGRAM is None:
        _PROGRAM = _build_program()
    return _PROGRAM


def _prep_inputs(x, w, q):
    """Slice/transpose full inputs into 8 per-core input maps."""
    from numpy.lib.stride_tricks import sliding_window_view

    np_dt = np.dtype(IO_DTYPE)
    # patches[c, oh, ow, ij] = x[0, c, oh+i, ow+j]
    patches = sliding_window_view(x[0], (SIDE, SIDE), axis=(1, 2)).reshape(
        CH, OUT, OUT, IJ
    )
    w = w.reshape(CH, OUT, OUT, NUM * IJ)
    q = q.reshape(CH, OUT, OUT, NUM * IJ)

    NCHUNK = CH * CHUNKS
    in_maps = []
    for k in range(NCORES):
        r0 = k * ROWS
        r1 = min(r0 + ROWS, OUT)
        nr = r1 - r0

        wk = np.zeros((CH, OUT, ROWS, NUM * IJ), np_dt)
        wk[:, :, :nr, :] = w[:, r0:r1].transpose(0, 2, 1, 3)
        qk = np.zeros((CH, OUT, ROWS, NUM * IJ), np_dt)
        qk[:, :, :nr, :] = q[:, r0:r1].transpose(0, 2, 1, 3)
        pk = np.zeros((CH, OUT, ROWS, IJ), np_dt)
        pk[:, :, :nr, :] = patches[:, r0:r1].transpose(0, 2, 1, 3)

        def chunked(a, inner):
            # (CH, OUT, ROWS, inner) -> (CH*CHUNKS, OUT, HROWS*inner)
            return (
                a.reshape(CH, OUT, CHUNKS, HROWS * inner)
                .transpose(0, 2, 1, 3)
                .reshape(NCHUNK, OUT, HROWS * inner)
            )

        in_maps.append(
            {
                "wt": chunked(wk, NUM * IJ),
                "qt": chunked(qk, NUM * IJ),
                "pt": chunked(pk, IJ),
            }
        )
    return in_maps


def _assemble_output(results):
    parts = []
    for k in range(NCORES):
        r0 = k * ROWS
        nr = min(r0 + ROWS, OUT) - r0
        ok = (
            results[k]["ot"]
            .reshape(CH, CHUNKS, OUT, HROWS, NUM)
            .transpose(0, 2, 1, 3, 4)
            .reshape(CH, OUT, ROWS, NUM)
        )
        parts.append(ok.transpose(0, 2, 1, 3)[:, :nr])  # (CH, nr, OUT, NUM)
    out = np.concatenate(parts, axis=1)  # (CH, OUT, OUT, NUM)
    return out[None].astype(np.float32)


def kernel(x, w, q):
    from concourse.bass_utils import run_bass_kernel_spmd

    nc = _get_program()
    in_maps = _prep_inputs(
        np.asarray(x, np.float32), np.asarray(w, np.float32), np.asarray(q, np.float32)
    )
    res = run_bass_kernel_spmd(nc, in_maps, list(range(NCORES)), trace=False)
    return _assemble_output(res.results)


def bench(x, w, q, iters=30):
    """Steady-state per-call wall time (ns) with device-resident inputs.

    Replicates bass2jax.run_bass_via_pjrt's multi-core path (shard_map over 8
    cores) but without output-buffer donation, so the jitted executable can be
    invoked repeatedly on the same device buffers.
    """
    import jax
    import numpy as _np
    from jax.sharding import Mesh, PartitionSpec
    from jax.experimental.shard_map import shard_map
    import concourse.mybir as mybir
    from concourse import bass2jax

    bass2jax.install_neuronx_cc_hook()
    nc = _get_program()
    in_maps = _prep_inputs(
        np.asarray(x, np.float32), np.asarray(w, np.float32), np.asarray(q, np.float32)
    )

    partition_name = nc.partition_id_tensor.name if nc.partition_id_tensor else None
    in_names, out_names, out_avals, zero_outs = [], [], [], []
    for alloc in nc.m.functions[0].allocations:
        if not isinstance(alloc, mybir.MemoryLocationSet):
            continue
        name = alloc.memorylocations[0].name
        if alloc.kind == "ExternalInput":
            if name != partition_name:
                in_names.append(name)
        elif alloc.kind == "ExternalOutput":
            out_names.append(name)
            shape = tuple(alloc.tensor_shape)
            dtype = mybir.dt.np(alloc.dtype)
            out_avals.append(jax.core.ShapedArray(shape, dtype))
            zero_outs.append(_np.zeros(shape, dtype))
    n_params = len(in_names)
    all_names = in_names + out_names
    if partition_name is not None:
        all_names = all_names + [partition_name]

    def _body(*args):
        operands = list(args)
        if partition_name is not None:
            operands.append(bass2jax.partition_id_tensor())
        outs = bass2jax._bass_exec_p.bind(
            *operands,
            out_avals=tuple(out_avals),
            in_names=tuple(all_names),
            out_names=tuple(out_names),
            lowering_input_output_aliases=(),
            sim_require_finite=True,
            sim_require_nnan=True,
            nc=nc,
        )
        return tuple(outs)

    devices = jax.devices()[:NCORES]
    mesh = Mesh(_np.asarray(devices), ("core",))
    nin = n_params + len(out_names)
    sharded = jax.jit(
        shard_map(
            _body,
            mesh=mesh,
            in_specs=(PartitionSpec("core"),) * nin,
            out_specs=(PartitionSpec("core"),) * len(out_names),
            check_rep=False,
        ),
        keep_unused=True,
    )
    concat_in = [
        _np.concatenate([in_maps[c][nm] for c in range(NCORES)], axis=0)
        for nm in in_names
    ]
    concat_zeros = [
        _np.zeros((NCORES * z.shape[0], *z.shape[1:]), z.dtype) for z in zero_outs
    ]
    args = [jax.device_put(a) for a in concat_in + concat_zeros]

    out = sharded(*args)  # compile + warmup
    jax.block_until_ready(out)
    times = []
    for _ in range(iters):
        t0 = time.perf_counter()
        out = sharded(*args)
        jax.block_until_ready(out)
        times.append(time.perf_counter() - t0)
    times.sort()
    med = times[len(times) // 2]
    print(
        f"bench: min {times[0] * 1e6:.1f} us, median {med * 1e6:.1f} us, "
        f"max {times[-1] * 1e6:.1f} us over {iters} iters"
    )
    return med * 1e9
